# revision 20
# baseline (speedup 1.0000x reference)
"""Trainium2 Bass kernel for nn_Attention_50654844289068.

Strategy (8 NeuronCores, data-parallel over batch B=8 -> 1 batch element per core):

  reference math per batch b:
    q = query @ Wq.T + bq            (S, 64)
    k = key   @ Wk.T + bk            (S, 64)
    v = value @ Wv.T + bv            (S, 64)
    s = (q @ k.T) * scale            (S, S)
    s = where(s == 0, eps, s); s = where(mask == 0, eps, s)
    w = softmax(s, axis=-1)          (S, S)   <- output 2
    att = w @ v                      (S, 64)  <- output 1

  Device-side layout choices (per core):
    - All big tensors are handled in TRANSPOSED score layout  sT[sk, sq]
      so that softmax's reduction axis (sk) lands on the partition axis,
      where the TensorEngine can reduce it for free via an appended
      ones-column in the attention@V matmul, and the e^T tiles are directly
      usable as the stationary operand of that matmul (no on-chip 2048x2048
      transpose needed).
    - The host pre-transposes query/key/value ( -> [512, S]) and the mask
      ( -> maskT[sk, sq]) while sharding, and post-transposes the weight
      output (device writes w^T).  Host-side work is only layout/dtype prep.
    - masked_fill(s==0, eps) + masked_fill(mask==0, eps):  eps = 1e-6, and
      exp(1e-6) == 1 + 1e-6.  We instead compute e = exp(scale*s*mask) so
      masked lanes give exp(0) = 1 — a 1e-6 relative difference, far below
      tolerance.  softmax has no max-subtraction: scores*scale are O(+-2),
      exp is perfectly stable there (matches jax softmax mathematically).
    - bf16 compute on PE (fp32 matmul is 4x slower), fp32 PSUM accumulate.

  Per-core phases:
    P0: project q^T,k^T [64,S] (bf16) and v [S,64] (+ ones col) from
        host-transposed inputs.
    P1: for each of 16 sk-blocks: scoresT = k_blk^T . q  (PE) ->
        s' = (scores*scale)*mask (DVE, reads PSUM) -> eT = exp(s') (ACT)
        -> accumulate attT[65, S] += [v|1]^T . eT (PE).
    P2: rowsums = attT row 64 -> 1/r (DVE) -> broadcast over partitions via
        a tiny DRAM bounce -> w^T = eT * rinv (DVE) -> DMA out;
        att = (attT rows 0:64 * rinv) transposed back on PE -> DMA out.
"""

import os
import sys
from contextlib import ExitStack

sys.path.insert(0, "/opt/trn_rl_repo")

import numpy as np
import ml_dtypes

import concourse.bacc as bacc
import concourse.bass as bass
import concourse.tile as tile
from concourse import masks, mybir
from concourse.bass_utils import run_bass_kernel_spmd

B, S, DM, DK = 8, 2048, 512, 64
NCORES = 8
P = 128
NKB = S // P          # 16 sk blocks
NCH = S // 512        # 4 sq chunks of 512
SCALE = float(DK) ** -0.5

F32 = mybir.dt.float32
BF16 = mybir.dt.bfloat16
U8 = mybir.dt.uint8
NPBF16 = ml_dtypes.bfloat16

AF = mybir.ActivationFunctionType
OP = mybir.AluOpType

LAST_RESULTS = None


def build_graph():
    nc = bacc.Bacc(
        "TRN2",
        target_bir_lowering=False,
        debug=False,
        num_devices=NCORES,
    )

    qT = nc.declare_dram_parameter("qT", [DM, S], BF16, isOutput=False)
    kT = nc.declare_dram_parameter("kT", [DM, S], BF16, isOutput=False)
    vT = nc.declare_dram_parameter("vT", [DM, S], BF16, isOutput=False)
    maskT = nc.declare_dram_parameter("maskT", [S, S], U8, isOutput=False)
    wTs = {
        t: nc.declare_dram_parameter(f"w{t}T", [DM, DK], BF16, isOutput=False)
        for t in "qkv"
    }
    biases = {
        t: nc.declare_dram_parameter(f"b{t}", [P, 1], F32, isOutput=False)
        for t in "qkv"
    }
    w_t = nc.declare_dram_parameter("w_t", [S, S], BF16, isOutput=True)
    att_t = nc.declare_dram_parameter("att_t", [DK, S], F32, isOutput=True)

    r_scr = nc.dram_tensor("r_scr", [S], F32)
    rinv_scr = nc.dram_tensor("rinv_scr", [S], BF16)

    xTs = {"q": qT, "k": kT, "v": vT}

    with tile.TileContext(nc) as tc, ExitStack() as ctx:
        persist = ctx.enter_context(tc.tile_pool(name="persist", bufs=1))
        et_pool = ctx.enter_context(tc.tile_pool(name="et", bufs=2 * NKB))

        ident_bf16 = persist.tile([P, P], BF16, tag="ident_bf16")
        masks.make_identity(nc, ident_bf16[:, :])

        # q^T and k^T duplicated onto both partition halves [0:64) and
        # [64:128) so consecutive sk blocks can run as concurrent row-group
        # matmul tiles on the PE array (K=64 uses only half the rows).
        qT_sb = persist.tile([P, S], BF16, tag="qT_sb")
        kT_sb = persist.tile([P, S], BF16, tag="kT_sb")
        v1_tiles = [
            persist.tile([P, DK + 1], BF16, tag=f"v1_{i}", name=f"v1_{i}")
            for i in range(NKB)
        ]

        # ---------------- P0: projections ----------------
        with tc.tile_pool(name="pro_in", bufs=4) as pro_in, \
             tc.tile_pool(name="pro_w", bufs=1) as pro_w, \
             tc.tile_pool(name="pro_tmp", bufs=1) as pro_tmp, \
             tc.tile_pool(name="pro_ps", bufs=4, space="PSUM") as pro_ps, \
             tc.tile_pool(name="tr_ps", bufs=2, space="PSUM") as tr_ps:

            bias_sb = {}
            for t in "qkv":
                bt = pro_w.tile([P, 1], F32, tag=f"bias_{t}")
                nc.sync.dma_start(bt[:, :], biases[t][:, :])
                bias_sb[t] = bt

            vT_sb = pro_tmp.tile([DK, S], BF16, tag="vT_sb")
            dst = {"q": qT_sb, "k": kT_sb, "v": vT_sb}

            for t in "qkv":
                dup = t in "qk"   # duplicate onto partitions [64:128)
                w_tiles = []
                for m in range(4):
                    wt = pro_w.tile([P, DK], BF16, tag=f"w_{t}_{m}")
                    nc.sync.dma_start(wt[:, :], wTs[t][m * P:(m + 1) * P, :])
                    w_tiles.append(wt)
                pss = [pro_ps.tile([P, 512], F32, tag="proj_ps", name=f"ps_{t}_{c}")
                       for c in range(NCH)]
                for m in range(4):
                    xt = pro_in.tile([P, S], BF16, tag="xin")
                    nc.sync.dma_start(xt[:, :], xTs[t][m * P:(m + 1) * P, :])
                    for c in range(NCH):
                        nc.tensor.matmul(
                            pss[c][0:DK, :],
                            w_tiles[m][:, :],
                            xt[:, c * 512:(c + 1) * 512],
                            start=(m == 0),
                            stop=(m == 3),
                            tile_position=(0, 0),
                            skip_group_check=True,
                        )
                        if dup:
                            nc.tensor.matmul(
                                pss[c][DK:2 * DK, :],
                                w_tiles[m][:, :],
                                xt[:, c * 512:(c + 1) * 512],
                                start=(m == 0),
                                stop=(m == 3),
                                tile_position=(0, DK),
                                skip_group_check=True,
                            )
                for c in range(NCH):
                    if dup:
                        nc.scalar.activation(
                            dst[t][:, c * 512:(c + 1) * 512],
                            pss[c][:, :],
                            AF.Identity,
                            bias=bias_sb[t][:, :],
                            scale=1.0,
                        )
                    else:
                        nc.scalar.activation(
                            dst[t][:, c * 512:(c + 1) * 512],
                            pss[c][0:DK, :],
                            AF.Identity,
                            bias=bias_sb[t][0:DK, :],
                            scale=1.0,
                        )

            # v natural [sk, 64] tiles with an appended ones column
            for i in range(NKB):
                pst = tr_ps.tile([P, DK], BF16, tag="tr_ps")
                nc.tensor.transpose(
                    pst[:, :], vT_sb[:, i * P:(i + 1) * P], ident_bf16[:DK, :DK]
                )
                nc.scalar.copy(v1_tiles[i][:, 0:DK], pst[:, :])
                nc.vector.memset(v1_tiles[i][:, DK:DK + 1], 1.0)


        # -------- main: flat cross-half pipeline of kb-pairs --------
        # Score matmuls for a kb pair are emitted adjacently (alternating PE
        # row groups -> they stream concurrently on the half-filled array)
        # and one pair AHEAD of the STT/exp/AV consumers.  The two sq halves
        # form one flat pipeline; half 0's normalize+writeout is distributed
        # between half 1's compute steps so neither PE nor the DMA queue
        # sees a serial epilogue flood mid-kernel.
        H = S // 2
        HCH = H // 512
        NPAIR = NKB // 2          # 8 pairs per half
        TOTAL = 2 * NPAIR         # 16 pair-steps overall
        PL = 1                    # pairs of score lookahead
        with tc.tile_pool(name="att_ps", bufs=1, space="PSUM") as att_ps, \
             tc.tile_pool(name="mask_p", bufs=6) as mask_p, \
             tc.tile_pool(name="sp_p", bufs=4) as sp_p, \
             tc.tile_pool(name="sc_ps", bufs=3, space="PSUM") as sc_ps, \
             tc.tile_pool(name="ph2", bufs=2) as ph2, \
             tc.tile_pool(name="w_p", bufs=6) as w_p:

            st = [
                {"att_acc": None, "e": {}, "ps": {}, "mask": {},
                 "att_sb": None, "rinv_bc": None, "w_done": 0}
                for _ in range(2)
            ]

            def emit_scores(h, pj):
                s = st[h]
                c0 = h * H
                if s["att_acc"] is None:
                    s["att_acc"] = [
                        att_ps.tile([DK + 1, 512], F32, tag=f"att_acc{c}",
                                    name=f"att_acc{h}_{c}")
                        for c in range(HCH)
                    ]
                for kb in (2 * pj, 2 * pj + 1):
                    mt = mask_p.tile([P, H], U8, tag="mask",
                                     name=f"mask_{h}_{kb}")
                    nc.sync.dma_start(
                        mt[:, :], maskT[kb * P:(kb + 1) * P, c0:c0 + H]
                    )
                    s["mask"][kb] = mt
                    s["ps"][kb] = sc_ps.tile([P, H], F32, tag="sc_ps",
                                             name=f"ps_{h}_{kb}")
                for cc in range(HCH):
                    for kb in (2 * pj, 2 * pj + 1):
                        rg = DK * (kb % 2)
                        nc.tensor.matmul(
                            s["ps"][kb][:, cc * 512:(cc + 1) * 512],
                            kT_sb[rg:rg + DK, kb * P:(kb + 1) * P],
                            qT_sb[rg:rg + DK,
                                  c0 + cc * 512:c0 + (cc + 1) * 512],
                            tile_position=(rg, 0),
                        )

            def emit_consume(h, pj):
                s = st[h]
                for kb in (2 * pj, 2 * pj + 1):
                    sp = sp_p.tile([P, H], BF16, tag="sp")
                    nc.vector.scalar_tensor_tensor(
                        sp[:, :], s["ps"].pop(kb)[:, :], SCALE,
                        s["mask"].pop(kb)[:, :],
                        op0=OP.mult, op1=OP.mult,
                    )
                    e = et_pool.tile([P, H], BF16, tag="et", name=f"e_{h}_{kb}")
                    nc.scalar.activation(e[:, :], sp[:, :], AF.Exp,
                                         bias=0.0, scale=1.0)
                    s["e"][kb] = e
                    for cc in range(HCH):
                        nc.tensor.matmul(
                            s["att_acc"][cc][:, :],
                            v1_tiles[kb][:, :],
                            e[:, cc * 512:(cc + 1) * 512],
                            start=(kb == 0),
                            stop=(kb == NKB - 1),
                        )

            def emit_rinv(h):
                # att PSUM -> SBUF, rowsums -> 1/r -> partition-broadcast
                s = st[h]
                c0 = h * H
                att_sb = ph2.tile([DK + 1, H], F32, tag="att_sb",
                                  name=f"att_sb_{h}")
                for cc in range(HCH):
                    nc.scalar.copy(att_sb[:, cc * 512:(cc + 1) * 512],
                                   s["att_acc"][cc][:, :])
                s["att_acc"] = None
                s["att_sb"] = att_sb
                rr_t = ph2.tile([P, H // P], F32, tag="rr_t",
                                name=f"rr_t_{h}")
                nc.sync.dma_start(rr_t[:, :], att_sb[DK:DK + 1, :])
                rr_inv = ph2.tile([P, H // P], BF16, tag="rr_inv",
                                  name=f"rr_inv_{h}")
                with nc.allow_low_precision(reason="bf16 1/rowsum within tol"):
                    nc.vector.reciprocal(rr_inv[:, :], rr_t[:, :])
                nc.sync.dma_start(
                    rinv_scr[c0:c0 + H].rearrange("(p f) -> p f", p=P),
                    rr_inv[:, :],
                )
                rinv_bc = ph2.tile([P, H], BF16, tag="rinv_bc",
                                   name=f"rinv_bc_{h}")
                nc.sync.dma_start(
                    rinv_bc[:, :],
                    rinv_scr[c0:c0 + H]
                    .rearrange("(a s) -> a s", a=1)
                    .to_broadcast((P, H)),
                )
                s["rinv_bc"] = rinv_bc

            def emit_w(h, n, eng=None):
                # emit up to n weight-normalize+writeout blocks for half h
                s = st[h]
                c0 = h * H
                eng = eng or nc.vector
                while n > 0 and s["w_done"] < NKB:
                    kb = s["w_done"]
                    wsb = w_p.tile([P, H], BF16, tag="wsb")
                    eng.tensor_tensor(
                        wsb[:, :], s["e"].pop(kb)[:, :], s["rinv_bc"][:, :],
                        op=OP.mult,
                    )
                    nc.sync.dma_start(
                        w_t[kb * P:(kb + 1) * P, c0:c0 + H], wsb[:, :]
                    )
                    s["w_done"] += 1
                    n -= 1

            def emit_att_out(h):
                s = st[h]
                c0 = h * H
                attn_sb = ph2.tile([DK, H], F32, tag="attn_sb",
                                   name=f"attn_sb_{h}")
                nc.vector.tensor_tensor(
                    attn_sb[:, :], s["att_sb"][0:DK, :],
                    s["rinv_bc"][0:DK, :], op=OP.mult,
                )
                nc.sync.dma_start(att_t[:, c0:c0 + H], attn_sb[:, :])

            from contextlib import contextmanager

            @contextmanager
            def low_priority(offset=100):
                # make these instructions LATE in the scheduler's heap so
                # they fill engine/DMA gaps instead of blocking the
                # score->exp->AV critical chain on the in-order engines.
                before = tc.cur_priority
                tc.cur_priority = before + offset
                try:
                    yield
                finally:
                    tc.cur_priority -= offset

            for gs in range(TOTAL + PL):
                if gs < TOTAL:
                    emit_scores(gs // NPAIR, gs % NPAIR)
                cs = gs - PL
                if 0 <= cs < TOTAL:
                    emit_consume(cs // NPAIR, cs % NPAIR)
                    if cs == NPAIR - 1:
                        emit_rinv(0)
                        with low_priority(30):
                            emit_att_out(0)
                            # half 0's normalize runs on the otherwise-idle
                            # GpSimd engine, overlapping half 1's compute
                            # without touching DVE's in-order STT chain.
                            emit_w(0, NKB, eng=nc.gpsimd)
            emit_rinv(1)
            emit_att_out(1)
            emit_w(1, NKB)

    nc.finalize()
    return nc


_CACHE = {}


def _get_graph():
    if "nc" not in _CACHE:
        _CACHE["nc"] = build_graph()
    return _CACHE["nc"]


def make_in_maps(query, key, value, attention_mask, Wq, bq, Wk, bk, Wv, bv):
    query = np.asarray(query)
    key = np.asarray(key)
    value = np.asarray(value)
    attention_mask = np.asarray(attention_mask)
    shared = {
        "wqT": np.ascontiguousarray(np.asarray(Wq, np.float32).T).astype(NPBF16),
        "wkT": np.ascontiguousarray(np.asarray(Wk, np.float32).T).astype(NPBF16),
        "wvT": np.ascontiguousarray(np.asarray(Wv, np.float32).T).astype(NPBF16),
        "bq": np.tile(np.asarray(bq, np.float32), 2).reshape(P, 1),
        "bk": np.tile(np.asarray(bk, np.float32), 2).reshape(P, 1),
        "bv": np.tile(np.asarray(bv, np.float32), 2).reshape(P, 1),
    }
    in_maps = []
    for b in range(B):
        in_maps.append(
            {
                "qT": np.ascontiguousarray(query[b].T).astype(NPBF16),
                "kT": np.ascontiguousarray(key[b].T).astype(NPBF16),
                "vT": np.ascontiguousarray(value[b].T).astype(NPBF16),
                "maskT": np.ascontiguousarray(attention_mask[b].T).astype(np.uint8),
                **shared,
            }
        )
    return in_maps


def kernel(query, key, value, attention_mask, Wq, bq, Wk, bk, Wv, bv):
    global LAST_RESULTS
    nc = _get_graph()
    in_maps = make_in_maps(
        query, key, value, attention_mask, Wq, bq, Wk, bk, Wv, bv
    )
    res = run_bass_kernel_spmd(nc, in_maps, core_ids=list(range(NCORES)))
    LAST_RESULTS = res
    att = np.stack(
        [
            np.asarray(res.results[c]["att_t"], np.float32).T
            for c in range(NCORES)
        ]
    )
    w = np.stack(
        [
            np.asarray(res.results[c]["w_t"]).astype(np.float32).T
            for c in range(NCORES)
        ]
    )
    return np.ascontiguousarray(att), np.ascontiguousarray(w)


# revision 21
# speedup vs baseline: 1.1718x; 1.1718x over previous
"""Trainium2 Bass kernel for nn_Attention_50654844289068.

Strategy (8 NeuronCores, data-parallel over batch B=8 -> 1 batch element per core):

  reference math per batch b:
    q = query @ Wq.T + bq            (S, 64)
    k = key   @ Wk.T + bk            (S, 64)
    v = value @ Wv.T + bv            (S, 64)
    s = (q @ k.T) * scale            (S, S)
    s = where(s == 0, eps, s); s = where(mask == 0, eps, s)
    w = softmax(s, axis=-1)          (S, S)   <- output 2
    att = w @ v                      (S, 64)  <- output 1

  Device-side layout choices (per core):
    - All big tensors are handled in TRANSPOSED score layout  sT[sk, sq]
      so that softmax's reduction axis (sk) lands on the partition axis,
      where the TensorEngine can reduce it for free via an appended
      ones-column in the attention@V matmul, and the e^T tiles are directly
      usable as the stationary operand of that matmul (no on-chip 2048x2048
      transpose needed).
    - The host pre-transposes query/key/value ( -> [512, S]) and the mask
      ( -> maskT[sk, sq]) while sharding, and post-transposes the weight
      output (device writes w^T).  Host-side work is only layout/dtype prep.
    - masked_fill(s==0, eps) + masked_fill(mask==0, eps):  eps = 1e-6, and
      exp(1e-6) == 1 + 1e-6.  We instead compute e = exp(scale*s*mask) so
      masked lanes give exp(0) = 1 — a 1e-6 relative difference, far below
      tolerance.  softmax has no max-subtraction: scores*scale are O(+-2),
      exp is perfectly stable there (matches jax softmax mathematically).
    - bf16 compute on PE (fp32 matmul is 4x slower), fp32 PSUM accumulate.

  Per-core phases:
    P0: project q^T,k^T [64,S] (bf16) and v [S,64] (+ ones col) from
        host-transposed inputs.
    P1: for each of 16 sk-blocks: scoresT = k_blk^T . q  (PE) ->
        s' = (scores*scale)*mask (DVE, reads PSUM) -> eT = exp(s') (ACT)
        -> accumulate attT[65, S] += [v|1]^T . eT (PE).
    P2: rowsums = attT row 64 -> 1/r (DVE) -> broadcast over partitions via
        a tiny DRAM bounce -> w^T = eT * rinv (DVE) -> DMA out;
        att = (attT rows 0:64 * rinv) transposed back on PE -> DMA out.
"""

import os
import sys
from contextlib import ExitStack

sys.path.insert(0, "/opt/trn_rl_repo")

import numpy as np
import ml_dtypes

import concourse.bacc as bacc
import concourse.bass as bass
import concourse.tile as tile
from concourse import masks, mybir
from concourse.bass_utils import run_bass_kernel_spmd

B, S, DM, DK = 8, 2048, 512, 64
NCORES = 8
P = 128
NKB = S // P          # 16 sk blocks
NCH = S // 512        # 4 sq chunks of 512
SCALE = float(DK) ** -0.5

F32 = mybir.dt.float32
BF16 = mybir.dt.bfloat16
U8 = mybir.dt.uint8
NPBF16 = ml_dtypes.bfloat16

AF = mybir.ActivationFunctionType
OP = mybir.AluOpType

LAST_RESULTS = None


def build_graph():
    nc = bacc.Bacc(
        "TRN2",
        target_bir_lowering=False,
        debug=False,
        num_devices=NCORES,
    )

    qT = nc.declare_dram_parameter("qT", [DM, S], BF16, isOutput=False)
    kT = nc.declare_dram_parameter("kT", [DM, S], BF16, isOutput=False)
    vT = nc.declare_dram_parameter("vT", [DM, S], BF16, isOutput=False)
    maskT = nc.declare_dram_parameter("maskT", [S, S], U8, isOutput=False)
    wTs = {
        t: nc.declare_dram_parameter(f"w{t}T", [DM, DK], BF16, isOutput=False)
        for t in "qkv"
    }
    biases = {
        t: nc.declare_dram_parameter(f"b{t}", [P, 1], F32, isOutput=False)
        for t in "qkv"
    }
    w_t = nc.declare_dram_parameter("w_t", [S, S], BF16, isOutput=True)
    att_t = nc.declare_dram_parameter("att_t", [DK, S], F32, isOutput=True)

    r_scr = nc.dram_tensor("r_scr", [S], F32)
    rinv_scr = nc.dram_tensor("rinv_scr", [S], BF16)

    xTs = {"q": qT, "k": kT, "v": vT}

    with tile.TileContext(nc) as tc, ExitStack() as ctx:
        persist = ctx.enter_context(tc.tile_pool(name="persist", bufs=1))
        et_pool = ctx.enter_context(tc.tile_pool(name="et", bufs=2 * NKB))

        ident_bf16 = persist.tile([P, P], BF16, tag="ident_bf16")
        masks.make_identity(nc, ident_bf16[:, :])

        # q^T and k^T duplicated onto both partition halves [0:64) and
        # [64:128) so consecutive sk blocks can run as concurrent row-group
        # matmul tiles on the PE array (K=64 uses only half the rows).
        qT_sb = persist.tile([P, S], BF16, tag="qT_sb")
        kT_sb = persist.tile([P, S], BF16, tag="kT_sb")
        v1_tiles = [
            persist.tile([P, DK + 1], BF16, tag=f"v1_{i}", name=f"v1_{i}")
            for i in range(NKB)
        ]

        # ---------------- P0: projections ----------------
        with tc.tile_pool(name="pro_in", bufs=4) as pro_in, \
             tc.tile_pool(name="pro_w", bufs=1) as pro_w, \
             tc.tile_pool(name="pro_tmp", bufs=1) as pro_tmp, \
             tc.tile_pool(name="pro_ps", bufs=4, space="PSUM") as pro_ps, \
             tc.tile_pool(name="tr_ps", bufs=2, space="PSUM") as tr_ps:

            bias_sb = {}
            for t in "qkv":
                bt = pro_w.tile([P, 1], F32, tag=f"bias_{t}")
                nc.sync.dma_start(bt[:, :], biases[t][:, :])
                bias_sb[t] = bt

            vT_sb = pro_tmp.tile([DK, S], BF16, tag="vT_sb")
            dst = {"q": qT_sb, "k": kT_sb, "v": vT_sb}

            for t in "qkv":
                dup = t in "qk"   # duplicate onto partitions [64:128)
                w_tiles = []
                for m in range(4):
                    wt = pro_w.tile([P, DK], BF16, tag=f"w_{t}_{m}")
                    nc.sync.dma_start(wt[:, :], wTs[t][m * P:(m + 1) * P, :])
                    w_tiles.append(wt)
                pss = [pro_ps.tile([P, 512], F32, tag="proj_ps", name=f"ps_{t}_{c}")
                       for c in range(NCH)]
                for m in range(4):
                    xt = pro_in.tile([P, S], BF16, tag="xin")
                    nc.sync.dma_start(xt[:, :], xTs[t][m * P:(m + 1) * P, :])
                    for c in range(NCH):
                        nc.tensor.matmul(
                            pss[c][0:DK, :],
                            w_tiles[m][:, :],
                            xt[:, c * 512:(c + 1) * 512],
                            start=(m == 0),
                            stop=(m == 3),
                            tile_position=(0, 0),
                            skip_group_check=True,
                        )
                        if dup:
                            nc.tensor.matmul(
                                pss[c][DK:2 * DK, :],
                                w_tiles[m][:, :],
                                xt[:, c * 512:(c + 1) * 512],
                                start=(m == 0),
                                stop=(m == 3),
                                tile_position=(0, DK),
                                skip_group_check=True,
                            )
                for c in range(NCH):
                    if dup:
                        nc.scalar.activation(
                            dst[t][:, c * 512:(c + 1) * 512],
                            pss[c][:, :],
                            AF.Identity,
                            bias=bias_sb[t][:, :],
                            scale=1.0,
                        )
                    else:
                        nc.scalar.activation(
                            dst[t][:, c * 512:(c + 1) * 512],
                            pss[c][0:DK, :],
                            AF.Identity,
                            bias=bias_sb[t][0:DK, :],
                            scale=1.0,
                        )

            # v natural [sk, 64] tiles with an appended ones column
            for i in range(NKB):
                pst = tr_ps.tile([P, DK], BF16, tag="tr_ps")
                nc.tensor.transpose(
                    pst[:, :], vT_sb[:, i * P:(i + 1) * P], ident_bf16[:DK, :DK]
                )
                nc.scalar.copy(v1_tiles[i][:, 0:DK], pst[:, :])
                nc.vector.memset(v1_tiles[i][:, DK:DK + 1], 1.0)


        # -------- main: flat cross-half pipeline of kb-pairs --------
        # Score matmuls for a kb pair are emitted adjacently (alternating PE
        # row groups -> they stream concurrently on the half-filled array)
        # and one pair AHEAD of the STT/exp/AV consumers.  The two sq halves
        # form one flat pipeline; half 0's normalize+writeout is distributed
        # between half 1's compute steps so neither PE nor the DMA queue
        # sees a serial epilogue flood mid-kernel.
        H = S // 2
        HCH = H // 512
        NPAIR = NKB // 2          # 8 pairs per half
        TOTAL = 2 * NPAIR         # 16 pair-steps overall
        PL = 1                    # pairs of score lookahead
        with tc.tile_pool(name="att_ps", bufs=1, space="PSUM") as att_ps, \
             tc.tile_pool(name="mask_p", bufs=6) as mask_p, \
             tc.tile_pool(name="sp_p", bufs=4) as sp_p, \
             tc.tile_pool(name="sc_ps", bufs=3, space="PSUM") as sc_ps, \
             tc.tile_pool(name="ph2", bufs=2) as ph2, \
             tc.tile_pool(name="w_p", bufs=6) as w_p:

            st = [
                {"att_acc": None, "e": {}, "ps": {}, "mask": {},
                 "att_sb": None, "rinv_bc": None, "w_done": 0}
                for _ in range(2)
            ]

            def emit_scores(h, pj):
                s = st[h]
                c0 = h * H
                if s["att_acc"] is None:
                    s["att_acc"] = [
                        att_ps.tile([DK + 1, 512], F32, tag=f"att_acc{c}",
                                    name=f"att_acc{h}_{c}")
                        for c in range(HCH)
                    ]
                for kb in (2 * pj, 2 * pj + 1):
                    mt = mask_p.tile([P, H], U8, tag="mask",
                                     name=f"mask_{h}_{kb}")
                    nc.sync.dma_start(
                        mt[:, :], maskT[kb * P:(kb + 1) * P, c0:c0 + H]
                    )
                    s["mask"][kb] = mt
                    s["ps"][kb] = sc_ps.tile([P, H], F32, tag="sc_ps",
                                             name=f"ps_{h}_{kb}")
                for cc in range(HCH):
                    for kb in (2 * pj, 2 * pj + 1):
                        rg = DK * (kb % 2)
                        nc.tensor.matmul(
                            s["ps"][kb][:, cc * 512:(cc + 1) * 512],
                            kT_sb[rg:rg + DK, kb * P:(kb + 1) * P],
                            qT_sb[rg:rg + DK,
                                  c0 + cc * 512:c0 + (cc + 1) * 512],
                            tile_position=(rg, 0),
                        )

            def emit_consume(h, pj):
                s = st[h]
                for kb in (2 * pj, 2 * pj + 1):
                    sp = sp_p.tile([P, H], BF16, tag="sp")
                    nc.vector.scalar_tensor_tensor(
                        sp[:, :], s["ps"].pop(kb)[:, :], SCALE,
                        s["mask"].pop(kb)[:, :],
                        op0=OP.mult, op1=OP.mult,
                    )
                    e = et_pool.tile([P, H], BF16, tag="et", name=f"e_{h}_{kb}")
                    nc.scalar.activation(e[:, :], sp[:, :], AF.Exp,
                                         bias=0.0, scale=1.0)
                    s["e"][kb] = e
                    for cc in range(HCH):
                        nc.tensor.matmul(
                            s["att_acc"][cc][:, :],
                            v1_tiles[kb][:, :],
                            e[:, cc * 512:(cc + 1) * 512],
                            start=(kb == 0),
                            stop=(kb == NKB - 1),
                        )

            def emit_rinv(h):
                # att PSUM -> SBUF, rowsums -> 1/r -> partition-broadcast
                s = st[h]
                c0 = h * H
                att_sb = ph2.tile([DK + 1, H], F32, tag="att_sb",
                                  name=f"att_sb_{h}")
                for cc in range(HCH):
                    nc.scalar.copy(att_sb[:, cc * 512:(cc + 1) * 512],
                                   s["att_acc"][cc][:, :])
                s["att_acc"] = None
                s["att_sb"] = att_sb
                rr_t = ph2.tile([P, H // P], F32, tag="rr_t",
                                name=f"rr_t_{h}")
                nc.sync.dma_start(rr_t[:, :], att_sb[DK:DK + 1, :])
                rr_inv = ph2.tile([P, H // P], BF16, tag="rr_inv",
                                  name=f"rr_inv_{h}")
                with nc.allow_low_precision(reason="bf16 1/rowsum within tol"):
                    nc.vector.reciprocal(rr_inv[:, :], rr_t[:, :])
                nc.sync.dma_start(
                    rinv_scr[c0:c0 + H].rearrange("(p f) -> p f", p=P),
                    rr_inv[:, :],
                )
                rinv_bc = ph2.tile([P, H], BF16, tag="rinv_bc",
                                   name=f"rinv_bc_{h}")
                nc.sync.dma_start(
                    rinv_bc[:, :],
                    rinv_scr[c0:c0 + H]
                    .rearrange("(a s) -> a s", a=1)
                    .to_broadcast((P, H)),
                )
                s["rinv_bc"] = rinv_bc

            def emit_w(h, n, eng=None):
                # emit up to n weight-normalize+writeout blocks for half h
                s = st[h]
                c0 = h * H
                eng = eng or nc.vector
                while n > 0 and s["w_done"] < NKB:
                    kb = s["w_done"]
                    wsb = w_p.tile([P, H], BF16, tag="wsb")
                    eng.tensor_tensor(
                        wsb[:, :], s["e"].pop(kb)[:, :], s["rinv_bc"][:, :],
                        op=OP.mult,
                    )
                    nc.sync.dma_start(
                        w_t[kb * P:(kb + 1) * P, c0:c0 + H], wsb[:, :]
                    )
                    s["w_done"] += 1
                    n -= 1

            def emit_att_out(h):
                s = st[h]
                c0 = h * H
                attn_sb = ph2.tile([DK, H], F32, tag="attn_sb",
                                   name=f"attn_sb_{h}")
                nc.vector.tensor_tensor(
                    attn_sb[:, :], s["att_sb"][0:DK, :],
                    s["rinv_bc"][0:DK, :], op=OP.mult,
                )
                nc.sync.dma_start(att_t[:, c0:c0 + H], attn_sb[:, :])

            from contextlib import contextmanager

            @contextmanager
            def low_priority(offset=4000):
                # make these instructions LATE in the scheduler's heap so
                # they fill engine/DMA gaps instead of blocking the
                # score->exp->AV critical chain on the in-order engines.
                before = tc.cur_priority
                tc.cur_priority = before + offset
                try:
                    yield
                finally:
                    tc.cur_priority -= offset

            for gs in range(TOTAL + PL):
                if gs < TOTAL:
                    emit_scores(gs // NPAIR, gs % NPAIR)
                cs = gs - PL
                if 0 <= cs < TOTAL:
                    emit_consume(cs // NPAIR, cs % NPAIR)
                    if cs == NPAIR - 1:
                        emit_rinv(0)
                        with low_priority():
                            emit_att_out(0)
                            emit_w(0, NKB)
            emit_rinv(1)
            emit_att_out(1)
            emit_w(1, NKB)

    nc.finalize()
    return nc


_CACHE = {}


def _get_graph():
    if "nc" not in _CACHE:
        _CACHE["nc"] = build_graph()
    return _CACHE["nc"]


def make_in_maps(query, key, value, attention_mask, Wq, bq, Wk, bk, Wv, bv):
    query = np.asarray(query)
    key = np.asarray(key)
    value = np.asarray(value)
    attention_mask = np.asarray(attention_mask)
    shared = {
        "wqT": np.ascontiguousarray(np.asarray(Wq, np.float32).T).astype(NPBF16),
        "wkT": np.ascontiguousarray(np.asarray(Wk, np.float32).T).astype(NPBF16),
        "wvT": np.ascontiguousarray(np.asarray(Wv, np.float32).T).astype(NPBF16),
        "bq": np.tile(np.asarray(bq, np.float32), 2).reshape(P, 1),
        "bk": np.tile(np.asarray(bk, np.float32), 2).reshape(P, 1),
        "bv": np.tile(np.asarray(bv, np.float32), 2).reshape(P, 1),
    }
    in_maps = []
    for b in range(B):
        in_maps.append(
            {
                "qT": np.ascontiguousarray(query[b].T).astype(NPBF16),
                "kT": np.ascontiguousarray(key[b].T).astype(NPBF16),
                "vT": np.ascontiguousarray(value[b].T).astype(NPBF16),
                "maskT": np.ascontiguousarray(attention_mask[b].T).astype(np.uint8),
                **shared,
            }
        )
    return in_maps


def kernel(query, key, value, attention_mask, Wq, bq, Wk, bk, Wv, bv):
    global LAST_RESULTS
    nc = _get_graph()
    in_maps = make_in_maps(
        query, key, value, attention_mask, Wq, bq, Wk, bk, Wv, bv
    )
    res = run_bass_kernel_spmd(nc, in_maps, core_ids=list(range(NCORES)))
    LAST_RESULTS = res
    att = np.stack(
        [
            np.asarray(res.results[c]["att_t"], np.float32).T
            for c in range(NCORES)
        ]
    )
    w = np.stack(
        [
            np.asarray(res.results[c]["w_t"]).astype(np.float32).T
            for c in range(NCORES)
        ]
    )
    return np.ascontiguousarray(att), np.ascontiguousarray(w)
